# revision 9
# baseline (speedup 1.0000x reference)
"""BEV camera-to-grid scatter-sum kernel for Trainium2 (8 NeuronCores).

Strategy:
  - Host (cheap, O(Np) index math): replicate the reference geometry bit-exactly
    (eager jax on CPU, f32) to get each frustum point's voxel id + kept mask.
  - Point-level compaction: only kept points (~27% here) are shipped, in
    spatial patch order. Tiles = 128 consecutive kept points.
  - For each tile, the host computes per-point "slot codes": the rank of the
    point's voxel among the tile's distinct voxels (chunked 32 at a time;
    tiles with >32 distinct voxels become multiple jobs over the same x tile).
  - Device (all heavy data work): for each job, stream the x tile
    [128pts, 80ch] (f16), build the one-hot segment matrix S [128, 32] on the
    Vector engine (is_equal against an iota constant), and compute
    out[32slots, 80ch] = S.T @ x on the Tensor engine with S as the stationary
    operand (f32 PSUM accumulate), 4 jobs col-packed per PSUM tile via
    tile_position. The Scalar/Vector engines copy PSUM->SBUF (f16) and the
    compressed per-tile voxel sums stream back to HBM.
  - Host: scatter the ~140k compressed rows (instead of 2M points) into the
    [B, NZ*C, NX, NY] grid in float64, cast to f32.

The job list is sharded evenly across the 8 cores (jobs are uniform cost);
every core runs the identical NEFF on its own packed slice. Env knobs:
BEV_DTYPE=f16|bf16|f32r|f32 (default f16), BEV_OUT=f16|f32 (default f16),
BEV_TRACE=1 to capture an NTFF profile (sets kernel.LAST_EXEC_NS).
"""

import sys
import os
import types
import math

sys.path.insert(0, "/opt/trn_rl_repo")

import numpy as np

# ---- static config (mirrors the nn.Module init_kwargs) ----
IMG_H, IMG_W = 256, 704
FH, FW = 32, 88
D, C = 118, 80
B, N = 1, 6
D0, D1 = 1.0, 60.0
NX, NY, NZ = 360, 360, 1
DXv = np.array([0.3, 0.3, 20.0], np.float32)
BXv = np.array([-54.0 + 0.15, -54.0 + 0.15, 0.0], np.float32)
ALPHA = 1.5

NPTS = B * N * D * FH * FW          # 1,993,728 points
NTILE = NPTS // 128                 # 15,576 tiles of 128 points
NCORES = 8
SLOTS = 32                          # distinct-voxel slots per job
JPB = 64                            # jobs per device block

LAST_EXEC_NS = None                 # set by kernel() for test harness use


# --------------------------------------------------------------------------
# NTFF profiling hook shim (this image's antenv lacks axon_hooks)
# --------------------------------------------------------------------------
def _install_ntff_hook():
    if "antenv.axon_hooks" in sys.modules:
        return
    mod = types.ModuleType("antenv.axon_hooks")
    mod._hook = None
    mod.set_axon_ntff_profile_hook = lambda h: setattr(mod, "_hook", h)
    mod.get_axon_ntff_profile_hook = lambda: mod._hook
    sys.modules["antenv.axon_hooks"] = mod
    try:
        import antenv
        antenv.axon_hooks = mod
    except ImportError:
        pass
    try:
        from trn_agent_boot.trn_boot import _ntff_profile_via_ctypes
        mod.set_axon_ntff_profile_hook(
            _ntff_profile_via_ctypes("/opt/axon/libaxon_pjrt.so")
        )
    except Exception:
        pass


# --------------------------------------------------------------------------
# Host geometry: bit-exact replica of the reference's index computation
# --------------------------------------------------------------------------
def _host_voxel_ids(camera2lidar, camera_intrinsics, img_aug_matrix,
                    lidar_aug_matrix, denorms):
    """Returns (idx [Np] int32 global voxel ids, kept [Np] bool)."""
    import jax
    import jax.numpy as jnp

    cpu = jax.devices("cpu")[0]

    def geom_fn(sensor2ego, intrin, ida, bda, den):
        Xs, Ys = np.meshgrid(np.linspace(0, IMG_W - 1, FW),
                             np.linspace(0, IMG_H - 1, FH))
        rays = np.stack([Xs, Ys, np.ones_like(Xs), np.ones_like(Xs)], -1)
        rays = jnp.asarray(rays.astype(np.float32))
        d = ((np.arange(D) / D) ** ALPHA).astype(np.float32)
        d = np.broadcast_to(d[:, None, None], (D, FH, FW))
        xg = np.broadcast_to(
            np.linspace(0, IMG_W - 1, FW, dtype=np.float32)[None, None, :],
            (D, FH, FW))
        yg = np.broadcast_to(
            np.linspace(0, IMG_H - 1, FH, dtype=np.float32)[None, :, None],
            (D, FH, FW))
        frustum = np.stack([xg, yg, d, np.ones_like(d)], -1).astype(np.float32)
        frustum = jnp.asarray(frustum)

        ego2sensor = jnp.linalg.inv(sensor2ego)
        O3 = ego2sensor[..., :3, 3]
        n = den[:, :3] / jnp.linalg.norm(den[:, :3], axis=-1, keepdims=True)
        n = n.reshape(B, N, 3)
        nP0 = jnp.sum(n * (O3 + D0 * n), -1)
        nP1 = jnp.sum(n * (O3 + D1 * n), -1)
        Minv = jnp.linalg.inv(intrin) @ jnp.linalg.inv(ida)
        r = jnp.einsum('hwk,bnlk->bnhwl', rays, Minv)[..., :3]
        dirs = r / jnp.linalg.norm(r, axis=-1, keepdims=True)
        ndir = jnp.einsum('bnc,bnhwc->bnhw', n, dirs)
        t0 = nP0[:, :, None, None] / ndir
        tdiff = t0 - nP1[:, :, None, None] / ndir
        z = (t0[:, :, None] - frustum[None, None, ..., 2] * tdiff[:, :, None]) \
            * dirs[..., 2][:, :, None]
        fx = jnp.broadcast_to(frustum[..., 0], (B, N, D, FH, FW))
        fy = jnp.broadcast_to(frustum[..., 1], (B, N, D, FH, FW))
        pts = jnp.stack([fx, fy, z, jnp.ones_like(z)], -1)
        pts = jnp.einsum('bndhwk,bnlk->bndhwl', pts, jnp.linalg.inv(ida))
        pts = jnp.concatenate([pts[..., :2] * pts[..., 2:3], pts[..., 2:]], -1)
        mat = bda[:, None] @ (sensor2ego @ jnp.linalg.inv(intrin))
        geom = jnp.einsum('bndhwk,bnlk->bndhwl', pts, mat)[..., :3]

        g = ((geom.reshape(NPTS, 3) - jnp.asarray(BXv - DXv / 2.0))
             / jnp.asarray(DXv)).astype(jnp.int32)
        kept = ((g[:, 0] >= 0) & (g[:, 0] < NX) & (g[:, 1] >= 0)
                & (g[:, 1] < NY) & (g[:, 2] >= 0) & (g[:, 2] < NZ))
        idx = (g[:, 2] * NX + g[:, 0]) * NY + g[:, 1]
        return idx, kept

    # Run EAGERLY (no jit): XLA fusion perturbs f32 rounding enough to flip
    # a handful of points across voxel boundaries vs the reference's eager
    # op-by-op execution. Bit-exact index agreement matters more than speed.
    with jax.default_device(cpu):
        idx, kept = geom_fn(jnp.asarray(camera2lidar),
                            jnp.asarray(camera_intrinsics),
                            jnp.asarray(img_aug_matrix),
                            jnp.asarray(lidar_aug_matrix),
                            jnp.asarray(denorms))
        idx = np.asarray(idx)
        kept = np.asarray(kept)
    return idx.astype(np.int64), np.asarray(kept)


# --------------------------------------------------------------------------
# Host: tile ranking and job construction (fully vectorized)
# --------------------------------------------------------------------------
def _build_jobs(v):
    """v: [Ntiles, 128] voxel id per point (-1 = padding/dropped).
    Per tile, rank each valid point's voxel among the tile's distinct
    voxels. Returns:
      job_tile  [J] int32   source tile id of each job
      job_codes [J, 128] f32  slot code per point (-1 = not in this job)
      job_ids   [J, SLOTS] int64  global voxel id per slot (-1 = empty)
    """
    NT = len(v)

    order = np.argsort(v, axis=1, kind="stable")
    sv = np.take_along_axis(v, order, axis=1)
    first = np.ones((NT, 128), dtype=bool)
    first[:, 1:] = sv[:, 1:] != sv[:, :-1]
    # dropped points (-1) sort first; exclude them from ranking
    valid_sorted = sv >= 0
    new_distinct = first & valid_sorted
    rank_sorted = np.cumsum(new_distinct, axis=1) - 1
    rank_sorted = np.where(valid_sorted, rank_sorted, -1)
    # scatter ranks back to natural point order
    rank = np.empty_like(rank_sorted)
    np.put_along_axis(rank, order, rank_sorted, axis=1)
    m = new_distinct.sum(axis=1)  # distinct voxels per tile

    keep_tile = np.nonzero(m > 0)[0]
    job_tile_l, job_codes_l, job_ids_l = [], [], []
    max_chunks = int(math.ceil(m.max() / SLOTS)) if len(keep_tile) else 1
    for c in range(max_chunks):
        sel = keep_tile[m[keep_tile] > c * SLOTS]
        if len(sel) == 0:
            break
        rc = rank[sel] - c * SLOTS
        codes = np.where((rc >= 0) & (rc < SLOTS), rc, -1).astype(np.float32)
        # distinct ids for this chunk: sorted distinct values ranked
        # [c*SLOTS, c*SLOTS+SLOTS)
        ids = np.full((len(sel), SLOTS), -1, dtype=np.int64)
        sv_sel = sv[sel]
        nd_sel = new_distinct[sel]
        rs_sel = rank_sorted[sel]
        rows, cols = np.nonzero(nd_sel)
        r_of = rs_sel[rows, cols] - c * SLOTS
        ok = (r_of >= 0) & (r_of < SLOTS)
        ids[rows[ok], r_of[ok]] = sv_sel[rows[ok], cols[ok]]
        job_tile_l.append(sel.astype(np.int32))
        job_codes_l.append(codes)
        job_ids_l.append(ids)

    if not job_tile_l:
        return (np.zeros(0, np.int32), np.zeros((0, 128), np.float32),
                np.zeros((0, SLOTS), np.int64))
    job_tile = np.concatenate(job_tile_l)
    job_codes = np.concatenate(job_codes_l)
    job_ids = np.concatenate(job_ids_l)
    return job_tile, job_codes, job_ids


# --------------------------------------------------------------------------
# Device kernel (built per nblocks, cached)
# --------------------------------------------------------------------------
_NC_CACHE = {}


def _build_device_kernel(nblocks, mm_dtype="f32r", out_dtype="f32"):
    """mm_dtype: 'f32' (exact, 2-pass PE), 'f32r' (1-pass reduced fp32),
    'bf16'/'f16' (x shipped 2-byte: halves DMA, enables col-packing),
    'f8e3'/'f8e4' (x shipped 1-byte: quarters DMA vs f32).
    out_dtype: 'f32' or 'f16' for the compressed result stream."""
    key = (nblocks, mm_dtype, out_dtype)
    if key in _NC_CACHE:
        return _NC_CACHE[key]
    import concourse.bass as bass
    import concourse.tile as tile
    from concourse import bacc, mybir

    f32 = mybir.dt.float32
    f16 = mybir.dt.float16
    bf16 = mybir.dt.bfloat16
    if mm_dtype == "bf16":
        xdt = bf16
    elif mm_dtype == "f16":
        xdt = mybir.dt.float16
    elif mm_dtype == "f8e3":
        xdt = mybir.dt.float8e3
    elif mm_dtype == "f8e4":
        xdt = mybir.dt.float8e4
    elif mm_dtype == "f32r":
        xdt = mybir.dt.float32r
    else:
        xdt = f32
    nc = bacc.Bacc("TRN2", target_bir_lowering=False, debug=False)
    # col-packing needs a narrow (<=2-byte) dtype on TRN2
    flip = mm_dtype in ("bf16", "f16", "f8e3", "f8e4")
    sdt = f16 if flip else xdt  # one-hot S matrix dtype (0/1 exact in f16)
    xpk = nc.dram_tensor("xpk", [nblocks, 128, JPB * C], xdt, kind="ExternalInput")
    codes = nc.dram_tensor("codes", [nblocks, 128, JPB], f16, kind="ExternalInput")
    iota = nc.dram_tensor("iota", [128, SLOTS], f16, kind="ExternalInput")
    # flip=True  out block layout: [128, (JPB//4)*C]; job t at partitions
    #            [32*(u%4), +32), free [C*((t//16)*4 + u//4), +C), u = t%16
    # flip=False out block layout: [C, JPB*SLOTS]; job t at free [SLOTS*t, +SLOTS)
    OW = (JPB // 4) * C if flip else JPB * SLOTS
    OP = 128 if flip else C
    odt = mybir.dt.float16 if out_dtype == "f16" else f32
    out = nc.dram_tensor("out", [nblocks, OP, OW], odt, kind="ExternalOutput")

    W = JPB * SLOTS

    with tile.TileContext(nc) as tc:
        with (
            tc.tile_pool(name="const", bufs=1) as const_pool,
            tc.tile_pool(name="xin", bufs=9) as xin_pool,
            tc.tile_pool(name="cin", bufs=6) as cin_pool,
            tc.tile_pool(name="smat", bufs=6) as s_pool,
            tc.tile_pool(name="psum", bufs=8, space="PSUM") as psum_pool,
            tc.tile_pool(name="outb", bufs=6) as out_pool,
        ):
            iota_t = const_pool.tile([128, SLOTS], f16)
            nc.sync.dma_start(iota_t[:], iota[:])

            for b in range(nblocks):
                xt = xin_pool.tile([128, JPB * C], xdt)
                nc.sync.dma_start(xt[:], xpk[b])
                ct = cin_pool.tile([128, JPB], f16)
                nc.gpsimd.dma_start(ct[:], codes[b])

                st = s_pool.tile([128, W], sdt)
                # S[p, t*SLOTS + j] = (iota[p, j] == codes[p, t])
                # split between DVE and GpSimd to unload the vector engine
                vs = int(os.environ.get("BEV_TTSPLIT", "48"))
                st_ap = st[:, :vs * SLOTS].rearrange("p (t j) -> p t j", j=SLOTS)
                iota_b = iota_t[:].unsqueeze(1).broadcast_to((128, vs, SLOTS))
                ct_b = ct[:, :vs].unsqueeze(2).broadcast_to((128, vs, SLOTS))
                nc.vector.tensor_tensor(st_ap, iota_b, ct_b,
                                        mybir.AluOpType.is_equal)
                if vs < JPB:
                    gs = JPB - vs
                    st_ap2 = st[:, vs * SLOTS:].rearrange(
                        "p (t j) -> p t j", j=SLOTS)
                    iota_b2 = iota_t[:].unsqueeze(1).broadcast_to(
                        (128, gs, SLOTS))
                    ct_b2 = ct[:, vs:].unsqueeze(2).broadcast_to(
                        (128, gs, SLOTS))
                    nc.gpsimd.tensor_tensor(st_ap2, iota_b2, ct_b2,
                                            mybir.AluOpType.is_equal)

                ob = out_pool.tile([OP, OW], odt)
                if flip:
                    # S stationary (cheap 32-col weight load); x streams.
                    # out[32, C] per job, 4 jobs col-packed per PSUM tile,
                    # 16 jobs per PSUM tile (one bank each).
                    nh = JPB // 16
                    POW = OW // nh
                    for h in range(nh):
                        ps = psum_pool.tile([128, POW], f32)
                        for u in range(16):
                            t = h * 16 + u
                            cg = u % 4
                            fs = u // 4
                            nc.tensor.matmul(
                                ps[32 * cg:32 * cg + 32, C * fs:C * fs + C],
                                st[:, t * SLOTS:(t + 1) * SLOTS],
                                xt[:, t * C:(t + 1) * C],
                                start=True, stop=True,
                                tile_position=(0, 32 * cg),
                            )
                        eng = os.environ.get("BEV_COPYSPLIT", "svsg")[h % 4]
                        if eng == "s":
                            nc.scalar.copy(ob[:, h * POW:(h + 1) * POW], ps[:])
                        elif eng == "g":
                            nc.gpsimd.tensor_copy(
                                ob[:, h * POW:(h + 1) * POW], ps[:])
                        else:
                            nc.vector.tensor_copy(
                                ob[:, h * POW:(h + 1) * POW], ps[:])
                else:
                    # x stationary; out[C, SLOTS] per job.
                    half = JPB // 2
                    for h in range(2):
                        ps = psum_pool.tile([C, W // 2], f32)
                        for u in range(half):
                            t = h * half + u
                            nc.tensor.matmul(
                                ps[:, u * SLOTS:(u + 1) * SLOTS],
                                xt[:, t * C:(t + 1) * C],
                                st[:, t * SLOTS:(t + 1) * SLOTS],
                                start=True, stop=True,
                            )
                        nc.scalar.copy(ob[:, h * (W // 2):(h + 1) * (W // 2)],
                                       ps[:])
                nc.scalar.dma_start(out[b], ob[:])

    nc.compile()
    _NC_CACHE[key] = nc
    return nc


# --------------------------------------------------------------------------
# Host: fp8 quantization with per-(tile,voxel)-run error diffusion.
# Each voxel's points within a tile are quantized sequentially, carrying the
# rounding residual into the next point, so the device-computed per-run SUM
# of quantized values tracks the exact sum to ~one final carry instead of
# sqrt(n) independent rounding errors.
# --------------------------------------------------------------------------
def _diffuse_quantize(xr, vt, f8dtype):
    """xr: [NT,128,C] f32 point features (tile order); vt: [NT,128] voxel ids.
    Returns [NT,128,C] in f8dtype."""
    NT = len(vt)
    order = np.argsort(vt, axis=1, kind="stable")
    sv = np.take_along_axis(vt, order, axis=1)
    same = np.zeros((NT, 128), bool)
    same[:, 1:] = (sv[:, 1:] == sv[:, :-1]) & (sv[:, 1:] >= 0)
    xs = np.take_along_axis(xr, order[:, :, None], axis=1)
    q8 = np.empty((NT, 128, C), dtype=f8dtype)
    carry = np.zeros((NT, C), np.float32)
    for j in range(128):
        carry[~same[:, j]] = 0.0
        v = xs[:, j] + carry
        qv8 = v.astype(f8dtype)
        qv = qv8.astype(np.float32)
        q8[:, j] = qv8
        carry = v - qv
    out = np.empty_like(q8)
    np.put_along_axis(out, order[:, :, None], q8, axis=1)
    return out


# --------------------------------------------------------------------------
# Main entry
# --------------------------------------------------------------------------
def kernel(x, camera2lidar, camera_intrinsics, img_aug_matrix,
           lidar_aug_matrix, denorms):
    global LAST_EXEC_NS
    _install_ntff_hook()
    from concourse import bass_utils

    x = np.asarray(x)
    idx, kept = _host_voxel_ids(camera2lidar, camera_intrinsics,
                                img_aug_matrix, lidar_aug_matrix, denorms)

    # point-level compaction: only kept points are ever shipped to the
    # device, in spatial patch order (8x11 pixel patches per (n,d) slab --
    # tighter BEV footprint per 128-point tile than raster order, so fewer
    # distinct voxels per tile). Tiles = groups of 128 consecutive kept points.
    perm = np.arange(NPTS).reshape(N * B, D, FH // 8, 8, FW // 11, 11) \
             .transpose(0, 1, 2, 4, 3, 5).reshape(-1)
    keep_pos = perm[kept[perm]]
    nk = len(keep_pos)
    NT = max(1, (nk + 127) // 128)
    vflat = np.full(NT * 128, -1, dtype=np.int64)
    vflat[:nk] = idx[keep_pos]
    vt = vflat.reshape(NT, 128)

    job_tile, job_codes, job_ids = _build_jobs(vt)
    J = len(job_tile)

    # shard jobs evenly across cores, pad to a multiple of JPB
    per_core = int(math.ceil(J / NCORES))
    nblocks = max(1, int(math.ceil(per_core / JPB)))
    T = nblocks * JPB

    mm_dtype = os.environ.get("BEV_DTYPE", "f8e3")
    import ml_dtypes
    xnp_dtype = np.float32
    if mm_dtype == "bf16":
        xnp_dtype = ml_dtypes.bfloat16
    elif mm_dtype == "f16":
        xnp_dtype = np.float16
    elif mm_dtype == "f8e3":
        xnp_dtype = ml_dtypes.float8_e3m4
    elif mm_dtype == "f8e4":
        xnp_dtype = ml_dtypes.float8_e4m3

    # gather kept rows once: [NT, 128, C]
    x2d = x.reshape(NPTS, C)
    xr32 = np.zeros((NT * 128, C), dtype=np.float32)
    xr32[:nk] = x2d[keep_pos]
    xr32 = xr32.reshape(NT, 128, C)
    if mm_dtype in ("f8e3", "f8e4"):
        xr = _diffuse_quantize(xr32, vt, xnp_dtype)
    else:
        xr = xr32.astype(xnp_dtype)

    iota_np = np.broadcast_to(
        np.arange(SLOTS, dtype=np.float16)[None, :], (128, SLOTS)
    ).copy()

    in_maps = []
    core_ids_list = []
    for k in range(NCORES):
        sl = slice(k * per_core, min((k + 1) * per_core, J))
        jt = job_tile[sl]
        jc = job_codes[sl]
        xp = np.zeros((T, 128, C), dtype=xnp_dtype)
        if len(jt):
            xp[:len(jt)] = xr[jt]
        cp = np.full((T, 128), -1.0, dtype=np.float16)
        if len(jc):
            cp[:len(jc)] = jc
        # block layout: [nblocks, 128, JPB*C] with job t of block b at
        # free offset t*C; codes [nblocks, 128, JPB]
        xp = xp.reshape(nblocks, JPB, 128, C).transpose(0, 2, 1, 3) \
               .reshape(nblocks, 128, JPB * C)
        cp = cp.reshape(nblocks, JPB, 128).transpose(0, 2, 1) \
               .reshape(nblocks, 128, JPB)
        in_maps.append({
            "xpk": np.ascontiguousarray(xp),
            "codes": np.ascontiguousarray(cp),
            "iota": iota_np,
        })
        core_ids_list.append(k)

    out_dtype = os.environ.get("BEV_OUT", "f16")
    nc = _build_device_kernel(nblocks, mm_dtype, out_dtype)
    res = bass_utils.run_bass_kernel_spmd(
        nc, in_maps, core_ids=core_ids_list,
        trace=bool(int(os.environ.get("BEV_TRACE", "0"))),
    )
    LAST_EXEC_NS = res.exec_time_ns

    # host combine (float64 accumulate)
    G = np.zeros((B * NZ * NX * NY, C), dtype=np.float64)
    for k in range(NCORES):
        sl = slice(k * per_core, min((k + 1) * per_core, J))
        nj = sl.stop - sl.start
        if nj == 0:
            continue
        o = res.results[k]["out"]
        if mm_dtype in ("bf16", "f16", "f8e3", "f8e4"):
            # [nblocks, 128, (JPB//4)*C]; job t: u=t%16 -> partitions
            # [32*(u%4), +32), free [C*((t//16)*4 + u//4), +C)
            o5 = o.reshape(nblocks, 4, SLOTS, JPB // 4, C)
            ts = np.arange(JPB)
            cgs = (ts % 16) % 4
            fss = (ts // 16) * 4 + (ts % 16) // 4
            o = o5[:, cgs, :, fss]        # [JPB, nblocks, SLOTS, C]
            o = o.transpose(1, 0, 2, 3).reshape(T, SLOTS, C)[:nj]
        else:
            # [nblocks, C, JPB*SLOTS]; job t at free [SLOTS*t, +SLOTS)
            o = o.reshape(nblocks, C, JPB, SLOTS).transpose(0, 2, 3, 1) \
                 .reshape(T, SLOTS, C)[:nj]
        ids = job_ids[sl]  # [nj, SLOTS]
        valid = ids >= 0
        flat_ids = ids[valid]
        flat_vals = o[valid].astype(np.float64)
        np.add.at(G, flat_ids, flat_vals)

    out = G.astype(np.float32).reshape(B, NZ, NX, NY, C)
    return np.ascontiguousarray(
        out.transpose(0, 1, 4, 2, 3).reshape(B, NZ * C, NX, NY)
    )



# revision 11
# speedup vs baseline: 1.0093x; 1.0093x over previous
"""BEV camera-to-grid scatter-sum kernel for Trainium2 (8 NeuronCores).

Strategy:
  - Host (cheap, O(Np) index math): replicate the reference geometry bit-exactly
    (eager jax on CPU, f32) to get each frustum point's voxel id + kept mask.
  - Point-level compaction: only kept points (~27% here) are shipped, in
    spatial patch order. Tiles = 128 consecutive kept points.
  - For each tile, the host computes per-point "slot codes": the rank of the
    point's voxel among the tile's distinct voxels (chunked 32 at a time;
    tiles with >32 distinct voxels become multiple jobs over the same x tile).
    Jobs with <=16 used slots form a separate "c16" class with half-width
    one-hot matrices and half-size output blocks.
  - x ships as fp8-e3m4 with per-(tile,voxel)-run error diffusion on the
    host, so each voxel-run's device SUM carries ~one rounding residual
    instead of sqrt(n) independent ones.
  - Device (all heavy data work): for each job, build the one-hot segment
    matrix S [128, w] on the Vector engine (is_equal against an iota
    constant), and compute out[w, 80] = S.T @ x on the Tensor engine with S
    stationary, 4 jobs col-packed per PSUM tile via tile_position. Scalar/
    Vector copy PSUM->SBUF (f16); compressed per-job voxel sums stream back.
  - Host: scatter the compressed rows into the [B, NZ*C, NX, NY] grid in
    float64, cast to f32.

The job list is sharded evenly across the 8 cores; every core runs the
identical NEFF on its own packed slice. Env knobs: BEV_DTYPE=f8e3|f8e4|f16
(default f8e3), BEV_CS16/BEV_CS32 copy-engine splits, BEV_TRACE=1 to
capture an NTFF profile (sets kernel.LAST_EXEC_NS).
"""

import sys
import os
import types
import math

sys.path.insert(0, "/opt/trn_rl_repo")

import numpy as np

# ---- static config (mirrors the nn.Module init_kwargs) ----
IMG_H, IMG_W = 256, 704
FH, FW = 32, 88
D, C = 118, 80
B, N = 1, 6
D0, D1 = 1.0, 60.0
NX, NY, NZ = 360, 360, 1
DXv = np.array([0.3, 0.3, 20.0], np.float32)
BXv = np.array([-54.0 + 0.15, -54.0 + 0.15, 0.0], np.float32)
ALPHA = 1.5

NPTS = B * N * D * FH * FW          # 1,993,728 points
NCORES = 8
SLOTS = 32                          # max distinct-voxel slots per job
W16 = 16                            # narrow-class slot width
JPB = 64                            # jobs per device block

LAST_EXEC_NS = None                 # set by kernel() for test harness use


# --------------------------------------------------------------------------
# NTFF profiling hook shim (this image's antenv lacks axon_hooks)
# --------------------------------------------------------------------------
def _install_ntff_hook():
    if "antenv.axon_hooks" in sys.modules:
        return
    mod = types.ModuleType("antenv.axon_hooks")
    mod._hook = None
    mod.set_axon_ntff_profile_hook = lambda h: setattr(mod, "_hook", h)
    mod.get_axon_ntff_profile_hook = lambda: mod._hook
    sys.modules["antenv.axon_hooks"] = mod
    try:
        import antenv
        antenv.axon_hooks = mod
    except ImportError:
        pass
    try:
        from trn_agent_boot.trn_boot import _ntff_profile_via_ctypes
        mod.set_axon_ntff_profile_hook(
            _ntff_profile_via_ctypes("/opt/axon/libaxon_pjrt.so")
        )
    except Exception:
        pass


# --------------------------------------------------------------------------
# Host geometry: bit-exact replica of the reference's index computation
# --------------------------------------------------------------------------
def _host_voxel_ids(camera2lidar, camera_intrinsics, img_aug_matrix,
                    lidar_aug_matrix, denorms):
    """Returns (idx [Np] int32 global voxel ids, kept [Np] bool)."""
    import jax
    import jax.numpy as jnp

    cpu = jax.devices("cpu")[0]

    def geom_fn(sensor2ego, intrin, ida, bda, den):
        Xs, Ys = np.meshgrid(np.linspace(0, IMG_W - 1, FW),
                             np.linspace(0, IMG_H - 1, FH))
        rays = np.stack([Xs, Ys, np.ones_like(Xs), np.ones_like(Xs)], -1)
        rays = jnp.asarray(rays.astype(np.float32))
        d = ((np.arange(D) / D) ** ALPHA).astype(np.float32)
        d = np.broadcast_to(d[:, None, None], (D, FH, FW))
        xg = np.broadcast_to(
            np.linspace(0, IMG_W - 1, FW, dtype=np.float32)[None, None, :],
            (D, FH, FW))
        yg = np.broadcast_to(
            np.linspace(0, IMG_H - 1, FH, dtype=np.float32)[None, :, None],
            (D, FH, FW))
        frustum = np.stack([xg, yg, d, np.ones_like(d)], -1).astype(np.float32)
        frustum = jnp.asarray(frustum)

        ego2sensor = jnp.linalg.inv(sensor2ego)
        O3 = ego2sensor[..., :3, 3]
        n = den[:, :3] / jnp.linalg.norm(den[:, :3], axis=-1, keepdims=True)
        n = n.reshape(B, N, 3)
        nP0 = jnp.sum(n * (O3 + D0 * n), -1)
        nP1 = jnp.sum(n * (O3 + D1 * n), -1)
        Minv = jnp.linalg.inv(intrin) @ jnp.linalg.inv(ida)
        r = jnp.einsum('hwk,bnlk->bnhwl', rays, Minv)[..., :3]
        dirs = r / jnp.linalg.norm(r, axis=-1, keepdims=True)
        ndir = jnp.einsum('bnc,bnhwc->bnhw', n, dirs)
        t0 = nP0[:, :, None, None] / ndir
        tdiff = t0 - nP1[:, :, None, None] / ndir
        z = (t0[:, :, None] - frustum[None, None, ..., 2] * tdiff[:, :, None]) \
            * dirs[..., 2][:, :, None]
        fx = jnp.broadcast_to(frustum[..., 0], (B, N, D, FH, FW))
        fy = jnp.broadcast_to(frustum[..., 1], (B, N, D, FH, FW))
        pts = jnp.stack([fx, fy, z, jnp.ones_like(z)], -1)
        pts = jnp.einsum('bndhwk,bnlk->bndhwl', pts, jnp.linalg.inv(ida))
        pts = jnp.concatenate([pts[..., :2] * pts[..., 2:3], pts[..., 2:]], -1)
        mat = bda[:, None] @ (sensor2ego @ jnp.linalg.inv(intrin))
        geom = jnp.einsum('bndhwk,bnlk->bndhwl', pts, mat)[..., :3]

        g = ((geom.reshape(NPTS, 3) - jnp.asarray(BXv - DXv / 2.0))
             / jnp.asarray(DXv)).astype(jnp.int32)
        kept = ((g[:, 0] >= 0) & (g[:, 0] < NX) & (g[:, 1] >= 0)
                & (g[:, 1] < NY) & (g[:, 2] >= 0) & (g[:, 2] < NZ))
        idx = (g[:, 2] * NX + g[:, 0]) * NY + g[:, 1]
        return idx, kept

    # Run EAGERLY (no jit): XLA fusion perturbs f32 rounding enough to flip
    # a handful of points across voxel boundaries vs the reference's eager
    # op-by-op execution. Bit-exact index agreement matters more than speed.
    with jax.default_device(cpu):
        idx, kept = geom_fn(jnp.asarray(camera2lidar),
                            jnp.asarray(camera_intrinsics),
                            jnp.asarray(img_aug_matrix),
                            jnp.asarray(lidar_aug_matrix),
                            jnp.asarray(denorms))
        idx = np.asarray(idx)
        kept = np.asarray(kept)
    return idx.astype(np.int64), np.asarray(kept)


# --------------------------------------------------------------------------
# Host: tile ranking and job construction (fully vectorized)
# --------------------------------------------------------------------------
def _build_jobs(v):
    """v: [Ntiles, 128] voxel id per point (-1 = padding/dropped).
    Per tile, rank each valid point's voxel among the tile's distinct
    voxels. Returns:
      job_tile  [J] int32   source tile id of each job
      job_codes [J, 128] f32  slot code per point (-1 = not in this job)
      job_ids   [J, SLOTS] int64  global voxel id per slot (-1 = empty)
    """
    NT = len(v)

    order = np.argsort(v, axis=1, kind="stable")
    sv = np.take_along_axis(v, order, axis=1)
    first = np.ones((NT, 128), dtype=bool)
    first[:, 1:] = sv[:, 1:] != sv[:, :-1]
    # dropped points (-1) sort first; exclude them from ranking
    valid_sorted = sv >= 0
    new_distinct = first & valid_sorted
    rank_sorted = np.cumsum(new_distinct, axis=1) - 1
    rank_sorted = np.where(valid_sorted, rank_sorted, -1)
    # scatter ranks back to natural point order
    rank = np.empty_like(rank_sorted)
    np.put_along_axis(rank, order, rank_sorted, axis=1)
    m = new_distinct.sum(axis=1)  # distinct voxels per tile

    keep_tile = np.nonzero(m > 0)[0]
    job_tile_l, job_codes_l, job_ids_l = [], [], []
    max_chunks = int(math.ceil(m.max() / SLOTS)) if len(keep_tile) else 1
    for c in range(max_chunks):
        sel = keep_tile[m[keep_tile] > c * SLOTS]
        if len(sel) == 0:
            break
        rc = rank[sel] - c * SLOTS
        codes = np.where((rc >= 0) & (rc < SLOTS), rc, -1).astype(np.float32)
        # distinct ids for this chunk: sorted distinct values ranked
        # [c*SLOTS, c*SLOTS+SLOTS)
        ids = np.full((len(sel), SLOTS), -1, dtype=np.int64)
        sv_sel = sv[sel]
        nd_sel = new_distinct[sel]
        rs_sel = rank_sorted[sel]
        rows, cols = np.nonzero(nd_sel)
        r_of = rs_sel[rows, cols] - c * SLOTS
        ok = (r_of >= 0) & (r_of < SLOTS)
        ids[rows[ok], r_of[ok]] = sv_sel[rows[ok], cols[ok]]
        job_tile_l.append(sel.astype(np.int32))
        job_codes_l.append(codes)
        job_ids_l.append(ids)

    if not job_tile_l:
        return (np.zeros(0, np.int32), np.zeros((0, 128), np.float32),
                np.zeros((0, SLOTS), np.int64))
    job_tile = np.concatenate(job_tile_l)
    job_codes = np.concatenate(job_codes_l)
    job_ids = np.concatenate(job_ids_l)
    return job_tile, job_codes, job_ids


# --------------------------------------------------------------------------
# Device kernel (built per block structure, cached)
# --------------------------------------------------------------------------
_NC_CACHE = {}


def _build_device_kernel(n16, n32, jb32, mm_dtype="f8e3", out_dtype="f16"):
    """Class-split program: n16 blocks of 64 16-slot jobs, then n32 blocks
    of 64 32-slot jobs (last c32 block trimmed to jb32 jobs, multiple of 16).
    mm_dtype: dtype x ships in ('f8e3'/'f8e4'/'f16'/'bf16').
    out_dtype: 'f16' or 'f32' for the compressed result stream."""
    cs16 = os.environ.get("BEV_CS16", "svsv")
    cs32 = os.environ.get("BEV_CS32", "svss")
    key = (n16, n32, jb32, mm_dtype, out_dtype, cs16, cs32)
    if key in _NC_CACHE:
        return _NC_CACHE[key]
    import concourse.bass as bass
    import concourse.tile as tile
    from concourse import bacc, mybir

    f32 = mybir.dt.float32
    f16 = mybir.dt.float16
    xdt = {"bf16": mybir.dt.bfloat16, "f16": f16,
           "f8e3": mybir.dt.float8e3, "f8e4": mybir.dt.float8e4}[mm_dtype]
    odt = f16 if out_dtype == "f16" else f32
    nc = bacc.Bacc("TRN2", target_bir_lowering=False, debug=False)

    OW = (JPB // 4) * C       # 1280 free bytes of out per full block
    xpk16 = codes16 = out16 = None
    if n16:
        xpk16 = nc.dram_tensor("xpk16", [n16, 128, JPB * C], xdt,
                               kind="ExternalInput")
        codes16 = nc.dram_tensor("codes16", [n16, 128, JPB], f16,
                                 kind="ExternalInput")
        out16 = nc.dram_tensor("out16", [n16, 64, OW], odt,
                               kind="ExternalOutput")
    xpk32 = nc.dram_tensor("xpk32", [n32, 128, JPB * C], xdt,
                           kind="ExternalInput")
    codes32 = nc.dram_tensor("codes32", [n32, 128, JPB], f16,
                             kind="ExternalInput")
    out32 = nc.dram_tensor("out32", [n32, 128, OW], odt,
                           kind="ExternalOutput")
    iota = nc.dram_tensor("iota", [128, SLOTS], f16, kind="ExternalInput")

    with tile.TileContext(nc) as tc:
        with (
            tc.tile_pool(name="const", bufs=1) as const_pool,
            tc.tile_pool(name="xin", bufs=6) as xin_pool,
            tc.tile_pool(name="cin", bufs=6) as cin_pool,
            tc.tile_pool(name="smat", bufs=6) as s_pool,
            tc.tile_pool(name="psum", bufs=8, space="PSUM") as psum_pool,
            tc.tile_pool(name="outb", bufs=6) as out_pool,
        ):
            iota_t = const_pool.tile([128, SLOTS], f16)
            nc.sync.dma_start(iota_t[:], iota[:])

            def do_block(xsrc, csrc, osrc, b, w, jb, csplit, packed_out):
                xt = xin_pool.tile([128, JPB * C], xdt)
                nc.sync.dma_start(xt[:, :jb * C], xsrc[b][:, :jb * C])
                ct = cin_pool.tile([128, JPB], f16)
                nc.gpsimd.dma_start(ct[:, :jb], csrc[b][:, :jb])

                st = s_pool.tile([128, JPB * w], f16)
                # S[p, t*w + j] = (iota[p, j] == codes[p, t])
                st_ap = st[:, :jb * w].rearrange("p (t j) -> p t j", j=w)
                iota_b = iota_t[:, :w].unsqueeze(1).broadcast_to((128, jb, w))
                ct_b = ct[:, :jb].unsqueeze(2).broadcast_to((128, jb, w))
                nc.vector.tensor_tensor(st_ap, iota_b, ct_b,
                                        mybir.AluOpType.is_equal)

                nh = jb // 16
                ob = out_pool.tile([128, nh * 4 * C], odt)
                for h in range(nh):
                    ps = psum_pool.tile([128, 4 * C], f32)
                    for u in range(16):
                        t = h * 16 + u
                        cg = u % 4
                        fs = u // 4
                        nc.tensor.matmul(
                            ps[32 * cg:32 * cg + w, C * fs:C * fs + C],
                            st[:, t * w:(t + 1) * w],
                            xt[:, t * C:(t + 1) * C],
                            start=True, stop=True,
                            tile_position=(0, 32 * cg),
                        )
                    eng = csplit[h % len(csplit)]
                    dst = ob[:, h * 4 * C:(h + 1) * 4 * C]
                    if eng == "s":
                        nc.scalar.copy(dst, ps[:])
                    else:
                        nc.vector.tensor_copy(dst, ps[:])
                if packed_out:
                    # ship only the 16 used rows of each 32-row col-group
                    for g in range(4):
                        nc.sync.dma_start(osrc[b][16 * g:16 * (g + 1)],
                                          ob[32 * g:32 * g + 16, :])
                else:
                    nc.sync.dma_start(osrc[b][:, :nh * 4 * C], ob[:])

            # interleave classes to spread vector-heavy c32 blocks between
            # DMA-heavy c16 blocks
            seq = []
            i16 = i32 = 0
            while i16 < n16 or i32 < n32:
                if i16 < n16:
                    seq.append(("c16", i16)); i16 += 1
                if i32 < n32:
                    seq.append(("c32", i32)); i32 += 1
            for kind, b in seq:
                if kind == "c16":
                    do_block(xpk16, codes16, out16, b, W16, JPB, cs16, True)
                else:
                    jb = JPB if b < n32 - 1 else jb32
                    do_block(xpk32, codes32, out32, b, SLOTS, jb, cs32, False)

    nc.compile()
    _NC_CACHE[key] = nc
    return nc


# --------------------------------------------------------------------------
# Host: fp8 quantization with per-(tile,voxel)-run error diffusion.
# Each voxel's points within a tile are quantized sequentially, carrying the
# rounding residual into the next point, so the device-computed per-run SUM
# of quantized values tracks the exact sum to ~one final carry instead of
# sqrt(n) independent rounding errors.
# --------------------------------------------------------------------------
def _diffuse_quantize(xr, vt, f8dtype):
    """xr: [NT,128,C] f32 point features (tile order); vt: [NT,128] voxel ids.
    Returns [NT,128,C] in f8dtype."""
    NT = len(vt)
    order = np.argsort(vt, axis=1, kind="stable")
    sv = np.take_along_axis(vt, order, axis=1)
    same = np.zeros((NT, 128), bool)
    same[:, 1:] = (sv[:, 1:] == sv[:, :-1]) & (sv[:, 1:] >= 0)
    xs = np.take_along_axis(xr, order[:, :, None], axis=1)
    q8 = np.empty((NT, 128, C), dtype=f8dtype)
    carry = np.zeros((NT, C), np.float32)
    for j in range(128):
        carry[~same[:, j]] = 0.0
        v = xs[:, j] + carry
        qv8 = v.astype(f8dtype)
        qv = qv8.astype(np.float32)
        q8[:, j] = qv8
        carry = v - qv
    out = np.empty_like(q8)
    np.put_along_axis(out, order[:, :, None], q8, axis=1)
    return out


# --------------------------------------------------------------------------
# Main entry
# --------------------------------------------------------------------------
def kernel(x, camera2lidar, camera_intrinsics, img_aug_matrix,
           lidar_aug_matrix, denorms):
    global LAST_EXEC_NS
    _install_ntff_hook()
    from concourse import bass_utils
    import ml_dtypes

    x = np.asarray(x)
    idx, kept = _host_voxel_ids(camera2lidar, camera_intrinsics,
                                img_aug_matrix, lidar_aug_matrix, denorms)

    # point-level compaction: only kept points are ever shipped to the
    # device, in spatial patch order (8x11 pixel patches per (n,d) slab --
    # tighter BEV footprint per 128-point tile than raster order, so fewer
    # distinct voxels per tile). Tiles = groups of 128 consecutive kept points.
    perm = np.arange(NPTS).reshape(N * B, D, FH // 8, 8, FW // 11, 11) \
             .transpose(0, 1, 2, 4, 3, 5).reshape(-1)
    keep_pos = perm[kept[perm]]
    nk = len(keep_pos)
    NT = max(1, (nk + 127) // 128)
    vflat = np.full(NT * 128, -1, dtype=np.int64)
    vflat[:nk] = idx[keep_pos]
    vt = vflat.reshape(NT, 128)

    job_tile, job_codes, job_ids = _build_jobs(vt)
    J = len(job_tile)
    used = (job_ids >= 0).sum(1)

    mm_dtype = os.environ.get("BEV_DTYPE", "f8e3")
    xnp_dtype = {"bf16": ml_dtypes.bfloat16, "f16": np.float16,
                 "f8e3": ml_dtypes.float8_e3m4,
                 "f8e4": ml_dtypes.float8_e4m3}[mm_dtype]

    # gather kept rows once: [NT, 128, C]
    x2d = x.reshape(NPTS, C)
    xr32 = np.zeros((NT * 128, C), dtype=np.float32)
    xr32[:nk] = x2d[keep_pos]
    xr32 = xr32.reshape(NT, 128, C)
    if mm_dtype in ("f8e3", "f8e4"):
        xr = _diffuse_quantize(xr32, vt, xnp_dtype)
    else:
        xr = xr32.astype(xnp_dtype)

    # ---- class split and per-core sharding ----
    c16_idx = np.nonzero(used <= W16)[0]
    c32_idx = np.nonzero(used > W16)[0]
    pc16 = int(math.ceil(len(c16_idx) / NCORES))
    pc32 = int(math.ceil(len(c32_idx) / NCORES))
    n16 = pc16 // JPB                   # full c16 blocks per core
    eff32 = pc32 + (pc16 - n16 * JPB)   # leftover c16 jobs ride in c32 blocks
    n32 = max(1, int(math.ceil(eff32 / JPB)))
    jb32 = eff32 - (n32 - 1) * JPB
    jb32 = int(math.ceil(jb32 / 16)) * 16   # PSUM-tile granularity
    T16 = n16 * JPB
    T32 = (n32 - 1) * JPB + jb32

    iota_np = np.broadcast_to(
        np.arange(SLOTS, dtype=np.float16)[None, :], (128, SLOTS)
    ).copy()

    def pack(job_list, nblocks, T):
        """job_list: global job indices, -1 = pad. Returns xp, cp."""
        xp = np.zeros((nblocks * JPB, 128, C), dtype=xnp_dtype)
        cp = np.full((nblocks * JPB, 128), -1.0, dtype=np.float16)
        real = job_list[job_list >= 0]
        pos = np.nonzero(job_list >= 0)[0]
        xp[pos] = xr[job_tile[real]]
        cp[pos] = job_codes[real]
        xp = xp.reshape(nblocks, JPB, 128, C).transpose(0, 2, 1, 3) \
               .reshape(nblocks, 128, JPB * C)
        cp = cp.reshape(nblocks, JPB, 128).transpose(0, 2, 1) \
               .reshape(nblocks, 128, JPB)
        return np.ascontiguousarray(xp), np.ascontiguousarray(cp)

    in_maps = []
    streams16, streams32 = [], []
    for k in range(NCORES):
        j16 = c16_idx[k * pc16:(k + 1) * pc16]
        j32 = c32_idx[k * pc32:(k + 1) * pc32]
        s16 = np.full(T16, -1, dtype=np.int64)
        s16[:min(T16, len(j16))] = j16[:T16]
        lo16 = j16[T16:]                       # leftover c16 -> c32 stream
        s32 = np.full(T32, -1, dtype=np.int64)
        mix = np.concatenate([lo16, j32])
        s32[:len(mix)] = mix
        streams16.append(s16)
        streams32.append(s32)
        im = {"iota": iota_np}
        if n16:
            xp16, cp16 = pack(s16, n16, T16)
            im["xpk16"] = xp16
            im["codes16"] = cp16
        xp32, cp32 = pack(s32, n32, T32)
        im["xpk32"] = xp32
        im["codes32"] = cp32
        in_maps.append(im)

    out_dtype = os.environ.get("BEV_OUT", "f16")
    nc = _build_device_kernel(n16, n32, jb32, mm_dtype, out_dtype)
    res = bass_utils.run_bass_kernel_spmd(
        nc, in_maps, core_ids=list(range(NCORES)),
        trace=bool(int(os.environ.get("BEV_TRACE", "0"))),
    )
    LAST_EXEC_NS = res.exec_time_ns

    # ---- host combine (float64 accumulate) ----
    # out block layouts: job t = h*16 + fs*4 + cg (h: psum tile, fs: free
    # slot, cg: col group); c32: [128, 1280] rows cg*32+slot; c16 packed:
    # [64, 1280] rows cg*16+slot.
    G = np.zeros((B * NZ * NX * NY, C), dtype=np.float64)

    def accumulate(o_jobs, stream, w):
        # o_jobs: [T, w, C] f64-castable; stream: global job ids (-1 pad)
        realm = stream >= 0
        ids = job_ids[stream[realm]][:, :w]
        vals = o_jobs[realm]
        valid = ids >= 0
        np.add.at(G, ids[valid], vals[valid].astype(np.float64))

    for k in range(NCORES):
        if n16:
            o16 = res.results[k]["out16"]     # [n16, 64, 1280]
            o16 = o16.reshape(n16, 4, 16, 4, 4, C) \
                     .transpose(0, 3, 4, 1, 2, 5).reshape(T16, W16, C)
            accumulate(o16, streams16[k], W16)
        o32 = res.results[k]["out32"]         # [n32, 128, 1280]
        o32 = o32.reshape(n32, 4, SLOTS, 4, 4, C) \
                 .transpose(0, 3, 4, 1, 2, 5).reshape(n32 * JPB, SLOTS, C)
        accumulate(o32[:T32], streams32[k], SLOTS)

    out = G.astype(np.float32).reshape(B, NZ, NX, NY, C)
    return np.ascontiguousarray(
        out.transpose(0, 1, 4, 2, 3).reshape(B, NZ * C, NX, NY)
    )


# revision 17
# speedup vs baseline: 1.0630x; 1.0532x over previous
"""BEV camera-to-grid scatter-sum kernel for Trainium2 (8 NeuronCores).

Strategy:
  - Host (cheap, O(Np) index math): replicate the reference geometry bit-exactly
    (eager jax on CPU, f32) to get each frustum point's voxel id + kept mask.
  - Point-level compaction: only kept points (~27% here) are shipped, in
    spatial patch order. Tiles = 128 consecutive kept points.
  - For each tile, the host computes per-point "slot codes": the rank of the
    point's voxel among the tile's distinct voxels (chunked 32 at a time;
    tiles with >32 distinct voxels become multiple jobs over the same x tile).
    Jobs with <=16 used slots form a separate "c16" class with half-width
    one-hot matrices and half-size output blocks.
  - x ships as fp8-e3m4 with per-(tile,voxel)-run error diffusion on the
    host, so each voxel-run's device SUM carries ~one rounding residual
    instead of sqrt(n) independent ones.
  - Device (all heavy data work): for each job, build the one-hot segment
    matrix S [128, w] on the Vector engine (is_equal against an iota
    constant), and compute out[w, 80] = S.T @ x on the Tensor engine with S
    stationary, 4 jobs col-packed per PSUM tile via tile_position. Scalar/
    Vector copy PSUM->SBUF (f16); compressed per-job voxel sums stream back.
  - Host: scatter the compressed rows into the [B, NZ*C, NX, NY] grid in
    float64, cast to f32.

The job list is sharded evenly across the 8 cores; every core runs the
identical NEFF on its own packed slice. Env knobs: BEV_DTYPE=f8e3|f8e4|f16
(default f8e3), BEV_CS16/BEV_CS32 copy-engine splits, BEV_TRACE=1 to
capture an NTFF profile (sets kernel.LAST_EXEC_NS).
"""

import sys
import os
import types
import math

sys.path.insert(0, "/opt/trn_rl_repo")

import numpy as np

# ---- static config (mirrors the nn.Module init_kwargs) ----
IMG_H, IMG_W = 256, 704
FH, FW = 32, 88
D, C = 118, 80
B, N = 1, 6
D0, D1 = 1.0, 60.0
NX, NY, NZ = 360, 360, 1
DXv = np.array([0.3, 0.3, 20.0], np.float32)
BXv = np.array([-54.0 + 0.15, -54.0 + 0.15, 0.0], np.float32)
ALPHA = 1.5

NPTS = B * N * D * FH * FW          # 1,993,728 points
NCORES = 8
SLOTS = 32                          # max distinct-voxel slots per job
W16 = 16                            # narrow-class slot width
JPB = 64                            # jobs per device block

LAST_EXEC_NS = None                 # set by kernel() for test harness use


# --------------------------------------------------------------------------
# NTFF profiling hook shim (this image's antenv lacks axon_hooks)
# --------------------------------------------------------------------------
def _install_ntff_hook():
    if "antenv.axon_hooks" in sys.modules:
        return
    mod = types.ModuleType("antenv.axon_hooks")
    mod._hook = None
    mod.set_axon_ntff_profile_hook = lambda h: setattr(mod, "_hook", h)
    mod.get_axon_ntff_profile_hook = lambda: mod._hook
    sys.modules["antenv.axon_hooks"] = mod
    try:
        import antenv
        antenv.axon_hooks = mod
    except ImportError:
        pass
    try:
        from trn_agent_boot.trn_boot import _ntff_profile_via_ctypes
        mod.set_axon_ntff_profile_hook(
            _ntff_profile_via_ctypes("/opt/axon/libaxon_pjrt.so")
        )
    except Exception:
        pass


# --------------------------------------------------------------------------
# Host geometry: bit-exact replica of the reference's index computation
# --------------------------------------------------------------------------
def _host_voxel_ids(camera2lidar, camera_intrinsics, img_aug_matrix,
                    lidar_aug_matrix, denorms):
    """Returns (idx [Np] int32 global voxel ids, kept [Np] bool)."""
    import jax
    import jax.numpy as jnp

    cpu = jax.devices("cpu")[0]

    def geom_fn(sensor2ego, intrin, ida, bda, den):
        Xs, Ys = np.meshgrid(np.linspace(0, IMG_W - 1, FW),
                             np.linspace(0, IMG_H - 1, FH))
        rays = np.stack([Xs, Ys, np.ones_like(Xs), np.ones_like(Xs)], -1)
        rays = jnp.asarray(rays.astype(np.float32))
        d = ((np.arange(D) / D) ** ALPHA).astype(np.float32)
        d = np.broadcast_to(d[:, None, None], (D, FH, FW))
        xg = np.broadcast_to(
            np.linspace(0, IMG_W - 1, FW, dtype=np.float32)[None, None, :],
            (D, FH, FW))
        yg = np.broadcast_to(
            np.linspace(0, IMG_H - 1, FH, dtype=np.float32)[None, :, None],
            (D, FH, FW))
        frustum = np.stack([xg, yg, d, np.ones_like(d)], -1).astype(np.float32)
        frustum = jnp.asarray(frustum)

        ego2sensor = jnp.linalg.inv(sensor2ego)
        O3 = ego2sensor[..., :3, 3]
        n = den[:, :3] / jnp.linalg.norm(den[:, :3], axis=-1, keepdims=True)
        n = n.reshape(B, N, 3)
        nP0 = jnp.sum(n * (O3 + D0 * n), -1)
        nP1 = jnp.sum(n * (O3 + D1 * n), -1)
        Minv = jnp.linalg.inv(intrin) @ jnp.linalg.inv(ida)
        r = jnp.einsum('hwk,bnlk->bnhwl', rays, Minv)[..., :3]
        dirs = r / jnp.linalg.norm(r, axis=-1, keepdims=True)
        ndir = jnp.einsum('bnc,bnhwc->bnhw', n, dirs)
        t0 = nP0[:, :, None, None] / ndir
        tdiff = t0 - nP1[:, :, None, None] / ndir
        z = (t0[:, :, None] - frustum[None, None, ..., 2] * tdiff[:, :, None]) \
            * dirs[..., 2][:, :, None]
        fx = jnp.broadcast_to(frustum[..., 0], (B, N, D, FH, FW))
        fy = jnp.broadcast_to(frustum[..., 1], (B, N, D, FH, FW))
        pts = jnp.stack([fx, fy, z, jnp.ones_like(z)], -1)
        pts = jnp.einsum('bndhwk,bnlk->bndhwl', pts, jnp.linalg.inv(ida))
        pts = jnp.concatenate([pts[..., :2] * pts[..., 2:3], pts[..., 2:]], -1)
        mat = bda[:, None] @ (sensor2ego @ jnp.linalg.inv(intrin))
        geom = jnp.einsum('bndhwk,bnlk->bndhwl', pts, mat)[..., :3]

        g = ((geom.reshape(NPTS, 3) - jnp.asarray(BXv - DXv / 2.0))
             / jnp.asarray(DXv)).astype(jnp.int32)
        kept = ((g[:, 0] >= 0) & (g[:, 0] < NX) & (g[:, 1] >= 0)
                & (g[:, 1] < NY) & (g[:, 2] >= 0) & (g[:, 2] < NZ))
        idx = (g[:, 2] * NX + g[:, 0]) * NY + g[:, 1]
        return idx, kept

    # Run EAGERLY (no jit): XLA fusion perturbs f32 rounding enough to flip
    # a handful of points across voxel boundaries vs the reference's eager
    # op-by-op execution. Bit-exact index agreement matters more than speed.
    with jax.default_device(cpu):
        idx, kept = geom_fn(jnp.asarray(camera2lidar),
                            jnp.asarray(camera_intrinsics),
                            jnp.asarray(img_aug_matrix),
                            jnp.asarray(lidar_aug_matrix),
                            jnp.asarray(denorms))
        idx = np.asarray(idx)
        kept = np.asarray(kept)
    return idx.astype(np.int64), np.asarray(kept)


# --------------------------------------------------------------------------
# Host: tile ranking and job construction (fully vectorized)
# --------------------------------------------------------------------------
def _build_jobs(v):
    """v: [Ntiles, 128] voxel id per point (-1 = padding/dropped).
    Per tile, rank each valid point's voxel among the tile's distinct
    voxels. Returns:
      job_tile  [J] int32   source tile id of each job
      job_codes [J, 128] f32  slot code per point (-1 = not in this job)
      job_ids   [J, SLOTS] int64  global voxel id per slot (-1 = empty)
    """
    NT = len(v)

    order = np.argsort(v, axis=1, kind="stable")
    sv = np.take_along_axis(v, order, axis=1)
    first = np.ones((NT, 128), dtype=bool)
    first[:, 1:] = sv[:, 1:] != sv[:, :-1]
    # dropped points (-1) sort first; exclude them from ranking
    valid_sorted = sv >= 0
    new_distinct = first & valid_sorted
    rank_sorted = np.cumsum(new_distinct, axis=1) - 1
    rank_sorted = np.where(valid_sorted, rank_sorted, -1)
    # scatter ranks back to natural point order
    rank = np.empty_like(rank_sorted)
    np.put_along_axis(rank, order, rank_sorted, axis=1)
    m = new_distinct.sum(axis=1)  # distinct voxels per tile

    keep_tile = np.nonzero(m > 0)[0]
    job_tile_l, job_codes_l, job_ids_l = [], [], []
    max_chunks = int(math.ceil(m.max() / SLOTS)) if len(keep_tile) else 1
    for c in range(max_chunks):
        sel = keep_tile[m[keep_tile] > c * SLOTS]
        if len(sel) == 0:
            break
        rc = rank[sel] - c * SLOTS
        codes = np.where((rc >= 0) & (rc < SLOTS), rc, -1).astype(np.float32)
        # distinct ids for this chunk: sorted distinct values ranked
        # [c*SLOTS, c*SLOTS+SLOTS)
        ids = np.full((len(sel), SLOTS), -1, dtype=np.int64)
        sv_sel = sv[sel]
        nd_sel = new_distinct[sel]
        rs_sel = rank_sorted[sel]
        rows, cols = np.nonzero(nd_sel)
        r_of = rs_sel[rows, cols] - c * SLOTS
        ok = (r_of >= 0) & (r_of < SLOTS)
        ids[rows[ok], r_of[ok]] = sv_sel[rows[ok], cols[ok]]
        job_tile_l.append(sel.astype(np.int32))
        job_codes_l.append(codes)
        job_ids_l.append(ids)

    if not job_tile_l:
        return (np.zeros(0, np.int32), np.zeros((0, 128), np.float32),
                np.zeros((0, SLOTS), np.int64))
    job_tile = np.concatenate(job_tile_l)
    job_codes = np.concatenate(job_codes_l)
    job_ids = np.concatenate(job_ids_l)
    return job_tile, job_codes, job_ids


# --------------------------------------------------------------------------
# Device kernel (built per block structure, cached)
# --------------------------------------------------------------------------
_NC_CACHE = {}


def _build_device_kernel(n16, n32, jb32, mm_dtype="f8e3", out_dtype="f16"):
    """Class-split program: n16 blocks of 64 16-slot jobs, then n32 blocks
    of 64 32-slot jobs (last c32 block trimmed to jb32 jobs, multiple of 16).
    mm_dtype: dtype x ships in ('f8e3'/'f8e4'/'f16'/'bf16').
    out_dtype: 'f16' or 'f32' for the compressed result stream."""
    cs16 = os.environ.get("BEV_CS16", "svsv")
    cs32 = os.environ.get("BEV_CS32", "svss")
    key = (n16, n32, jb32, mm_dtype, out_dtype, cs16, cs32)
    if key in _NC_CACHE:
        return _NC_CACHE[key]
    import concourse.bass as bass
    import concourse.tile as tile
    from concourse import bacc, mybir

    f32 = mybir.dt.float32
    f16 = mybir.dt.float16
    xdt = {"bf16": mybir.dt.bfloat16, "f16": f16,
           "f8e3": mybir.dt.float8e3, "f8e4": mybir.dt.float8e4}[mm_dtype]
    odt = f16 if out_dtype == "f16" else f32
    nc = bacc.Bacc("TRN2", target_bir_lowering=False, debug=False)

    OW = (JPB // 4) * C       # 1280 free elems of out per full block
    T16 = n16 * JPB
    T32 = (n32 - 1) * JPB + jb32
    xpk16 = codes16 = out16 = None
    if n16:
        xpk16 = nc.dram_tensor("xpk16", [n16, 128, JPB * C], xdt,
                               kind="ExternalInput")
        codes16 = nc.dram_tensor("codes16", [128, T16], f16,
                                 kind="ExternalInput")
        out16 = nc.dram_tensor("out16", [64, n16 * OW], odt,
                               kind="ExternalOutput")
    xpk32 = nc.dram_tensor("xpk32", [n32, 128, JPB * C], xdt,
                           kind="ExternalInput")
    codes32 = nc.dram_tensor("codes32", [128, T32], f16,
                             kind="ExternalInput")
    out32 = nc.dram_tensor("out32", [n32, 128, OW], odt,
                           kind="ExternalOutput")
    iota = nc.dram_tensor("iota", [128, SLOTS], f16, kind="ExternalInput")

    with tile.TileContext(nc) as tc:
        with (
            tc.tile_pool(name="const", bufs=1) as const_pool,
            tc.tile_pool(name="xin", bufs=6) as xin_pool,
            tc.tile_pool(name="smat", bufs=6) as s_pool,
            tc.tile_pool(name="psum", bufs=8, space="PSUM") as psum_pool,
        ):
            iota_t = const_pool.tile([128, SLOTS], f16)
            nc.gpsimd.dma_start(iota_t[:], iota[:])
            # all codes land upfront in two batched DMAs
            ct16 = ct32 = None
            if n16:
                ct16 = const_pool.tile([128, T16], f16)
                nc.gpsimd.dma_start(ct16[:], codes16[:])
                ob16 = const_pool.tile([128, n16 * OW], odt)
            ct32 = const_pool.tile([128, T32], f16)
            nc.gpsimd.dma_start(ct32[:], codes32[:])
            ob32 = const_pool.tile([128, n32 * OW], odt)

            xeng = [nc.sync, nc.sync]

            def do_block(i, xsrc, ct_all, ob_all, b, w, jb, csplit):
                xt = xin_pool.tile([128, JPB * C], xdt)
                xeng[i % 2].dma_start(xt[:, :jb * C], xsrc[b][:, :jb * C])
                ct = ct_all[:, b * JPB:b * JPB + jb]

                st = s_pool.tile([128, JPB * w], f16)
                # S[p, t*w + j] = (iota[p, j] == codes[p, t])
                st_ap = st[:, :jb * w].rearrange("p (t j) -> p t j", j=w)
                iota_b = iota_t[:, :w].unsqueeze(1).broadcast_to((128, jb, w))
                ct_b = ct.unsqueeze(2).broadcast_to((128, jb, w))
                nc.vector.tensor_tensor(st_ap, iota_b, ct_b,
                                        mybir.AluOpType.is_equal)

                nh = jb // 16
                for h in range(nh):
                    ps = psum_pool.tile([128, 4 * C], f32)
                    for u in range(16):
                        t = h * 16 + u
                        cg = u % 4
                        fs = u // 4
                        nc.tensor.matmul(
                            ps[32 * cg:32 * cg + w, C * fs:C * fs + C],
                            st[:, t * w:(t + 1) * w],
                            xt[:, t * C:(t + 1) * C],
                            start=True, stop=True,
                            tile_position=(0, 32 * cg),
                        )
                    eng = csplit[h % len(csplit)]
                    dst = ob_all[:, b * OW + h * 4 * C:b * OW + (h + 1) * 4 * C]
                    if eng == "s":
                        nc.scalar.copy(dst, ps[:])
                    else:
                        nc.vector.tensor_copy(dst, ps[:])

            # interleave classes (spreads vector-heavy c32 blocks between
            # DMA-heavy c16 blocks), ending on a c32 block so the batched
            # out16 DMAs overlap the tail
            seq = []
            i16 = i32 = 0
            while i16 < n16 or i32 < n32:
                if i16 < n16:
                    seq.append(("c16", i16)); i16 += 1
                if i32 < n32:
                    seq.append(("c32", i32)); i32 += 1
            if len(seq) >= 2 and seq[-1][0] == "c16" and n32:
                seq[-1], seq[-2] = seq[-2], seq[-1]
            oeng = [nc.scalar, nc.gpsimd]
            n_emitted16 = 0
            for i, (kind, b) in enumerate(seq):
                if kind == "c16":
                    do_block(i, xpk16, ct16, ob16, b, W16, JPB, cs16)
                    n_emitted16 += 1
                    if n_emitted16 == n16:
                        # ship the 16 used rows of each 32-row col-group,
                        # all c16 blocks in one go per group
                        for g in range(4):
                            oeng[g % 2].dma_start(
                                out16[16 * g:16 * (g + 1)],
                                ob16[32 * g:32 * g + 16, :])
                else:
                    jb = JPB if b < n32 - 1 else jb32
                    do_block(i, xpk32, ct32, ob32, b, SLOTS, jb, cs32)
                    nc.scalar.dma_start(
                        out32[b][:, :(jb // 16) * 4 * C],
                        ob32[:, b * OW:b * OW + (jb // 16) * 4 * C])

    nc.compile()
    _NC_CACHE[key] = nc
    return nc


# --------------------------------------------------------------------------
# Host: fp8 quantization with per-(tile,voxel)-run error diffusion.
# Each voxel's points within a tile are quantized sequentially, carrying the
# rounding residual into the next point, so the device-computed per-run SUM
# of quantized values tracks the exact sum to ~one final carry instead of
# sqrt(n) independent rounding errors.
# --------------------------------------------------------------------------
def _diffuse_quantize(xr, vt, f8dtype):
    """xr: [NT,128,C] f32 point features (tile order); vt: [NT,128] voxel ids.
    Returns [NT,128,C] in f8dtype."""
    NT = len(vt)
    order = np.argsort(vt, axis=1, kind="stable")
    sv = np.take_along_axis(vt, order, axis=1)
    same = np.zeros((NT, 128), bool)
    same[:, 1:] = (sv[:, 1:] == sv[:, :-1]) & (sv[:, 1:] >= 0)
    xs = np.take_along_axis(xr, order[:, :, None], axis=1)
    q8 = np.empty((NT, 128, C), dtype=f8dtype)
    carry = np.zeros((NT, C), np.float32)
    for j in range(128):
        carry[~same[:, j]] = 0.0
        v = xs[:, j] + carry
        qv8 = v.astype(f8dtype)
        qv = qv8.astype(np.float32)
        q8[:, j] = qv8
        carry = v - qv
    out = np.empty_like(q8)
    np.put_along_axis(out, order[:, :, None], q8, axis=1)
    return out


# --------------------------------------------------------------------------
# Main entry
# --------------------------------------------------------------------------
def kernel(x, camera2lidar, camera_intrinsics, img_aug_matrix,
           lidar_aug_matrix, denorms):
    global LAST_EXEC_NS
    _install_ntff_hook()
    from concourse import bass_utils
    import ml_dtypes

    x = np.asarray(x)
    idx, kept = _host_voxel_ids(camera2lidar, camera_intrinsics,
                                img_aug_matrix, lidar_aug_matrix, denorms)

    # point-level compaction: only kept points are ever shipped to the
    # device, in spatial patch order (8x11 pixel patches per (n,d) slab --
    # tighter BEV footprint per 128-point tile than raster order, so fewer
    # distinct voxels per tile). Tiles = groups of 128 consecutive kept points.
    perm = np.arange(NPTS).reshape(N * B, D, FH // 8, 8, FW // 11, 11) \
             .transpose(0, 1, 2, 4, 3, 5).reshape(-1)
    keep_pos = perm[kept[perm]]
    nk = len(keep_pos)
    NT = max(1, (nk + 127) // 128)
    vflat = np.full(NT * 128, -1, dtype=np.int64)
    vflat[:nk] = idx[keep_pos]
    vt = vflat.reshape(NT, 128)

    job_tile, job_codes, job_ids = _build_jobs(vt)
    J = len(job_tile)
    used = (job_ids >= 0).sum(1)

    mm_dtype = os.environ.get("BEV_DTYPE", "f8e3")
    xnp_dtype = {"bf16": ml_dtypes.bfloat16, "f16": np.float16,
                 "f8e3": ml_dtypes.float8_e3m4,
                 "f8e4": ml_dtypes.float8_e4m3}[mm_dtype]

    # gather kept rows once: [NT, 128, C]
    x2d = x.reshape(NPTS, C)
    xr32 = np.zeros((NT * 128, C), dtype=np.float32)
    xr32[:nk] = x2d[keep_pos]
    xr32 = xr32.reshape(NT, 128, C)
    if mm_dtype in ("f8e3", "f8e4"):
        xr = _diffuse_quantize(xr32, vt, xnp_dtype)
    else:
        xr = xr32.astype(xnp_dtype)

    # ---- class split and per-core sharding ----
    c16_idx = np.nonzero(used <= W16)[0]
    c32_idx = np.nonzero(used > W16)[0]
    pc16 = int(math.ceil(len(c16_idx) / NCORES))
    pc32 = int(math.ceil(len(c32_idx) / NCORES))
    n16 = pc16 // JPB                   # full c16 blocks per core
    eff32 = pc32 + (pc16 - n16 * JPB)   # leftover c16 jobs ride in c32 blocks
    n32 = max(1, int(math.ceil(eff32 / JPB)))
    jb32 = eff32 - (n32 - 1) * JPB
    jb32 = int(math.ceil(jb32 / 16)) * 16   # PSUM-tile granularity
    T16 = n16 * JPB
    T32 = (n32 - 1) * JPB + jb32

    iota_np = np.broadcast_to(
        np.arange(SLOTS, dtype=np.float16)[None, :], (128, SLOTS)
    ).copy()

    def pack(job_list, nblocks, T):
        """job_list: global job indices, -1 = pad. Returns xp [nb,128,JPB*C]
        and cp [128, T] (codes for all blocks, partition-major)."""
        xp = np.zeros((nblocks * JPB, 128, C), dtype=xnp_dtype)
        cp = np.full((nblocks * JPB, 128), -1.0, dtype=np.float16)
        real = job_list[job_list >= 0]
        pos = np.nonzero(job_list >= 0)[0]
        xp[pos] = xr[job_tile[real]]
        cp[pos] = job_codes[real]
        xp = xp.reshape(nblocks, JPB, 128, C).transpose(0, 2, 1, 3) \
               .reshape(nblocks, 128, JPB * C)
        cp = cp[:T].T
        return np.ascontiguousarray(xp), np.ascontiguousarray(cp)

    in_maps = []
    streams16, streams32 = [], []
    for k in range(NCORES):
        j16 = c16_idx[k * pc16:(k + 1) * pc16]
        j32 = c32_idx[k * pc32:(k + 1) * pc32]
        s16 = np.full(T16, -1, dtype=np.int64)
        s16[:min(T16, len(j16))] = j16[:T16]
        lo16 = j16[T16:]                       # leftover c16 -> c32 stream
        s32 = np.full(T32, -1, dtype=np.int64)
        mix = np.concatenate([lo16, j32])
        s32[:len(mix)] = mix
        streams16.append(s16)
        streams32.append(s32)
        im = {"iota": iota_np}
        if n16:
            xp16, cp16 = pack(s16, n16, T16)
            im["xpk16"] = xp16
            im["codes16"] = cp16
        xp32, cp32 = pack(s32, n32, T32)
        im["xpk32"] = xp32
        im["codes32"] = cp32
        in_maps.append(im)

    out_dtype = os.environ.get("BEV_OUT", "f16")
    nc = _build_device_kernel(n16, n32, jb32, mm_dtype, out_dtype)
    res = bass_utils.run_bass_kernel_spmd(
        nc, in_maps, core_ids=list(range(NCORES)),
        trace=bool(int(os.environ.get("BEV_TRACE", "0"))),
    )
    LAST_EXEC_NS = res.exec_time_ns

    # ---- host combine (float64 accumulate) ----
    # out block layouts: job t = h*16 + fs*4 + cg (h: psum tile, fs: free
    # slot, cg: col group); c32: [128, 1280] rows cg*32+slot; c16 packed:
    # [64, 1280] rows cg*16+slot.
    G = np.zeros((B * NZ * NX * NY, C), dtype=np.float64)

    def accumulate(o_jobs, stream, w):
        # o_jobs: [T, w, C] f64-castable; stream: global job ids (-1 pad)
        realm = stream >= 0
        ids = job_ids[stream[realm]][:, :w]
        vals = o_jobs[realm]
        valid = ids >= 0
        np.add.at(G, ids[valid], vals[valid].astype(np.float64))

    for k in range(NCORES):
        if n16:
            o16 = res.results[k]["out16"]     # [64, n16*1280]
            o16 = o16.reshape(4, W16, n16, 4, 4, C) \
                     .transpose(2, 3, 4, 0, 1, 5).reshape(T16, W16, C)
            accumulate(o16, streams16[k], W16)
        o32 = res.results[k]["out32"]         # [n32, 128, 1280]
        o32 = o32.reshape(n32, 4, SLOTS, 4, 4, C) \
                 .transpose(0, 3, 4, 1, 2, 5).reshape(n32 * JPB, SLOTS, C)
        accumulate(o32[:T32], streams32[k], SLOTS)

    out = G.astype(np.float32).reshape(B, NZ, NX, NY, C)
    return np.ascontiguousarray(
        out.transpose(0, 1, 4, 2, 3).reshape(B, NZ * C, NX, NY)
    )
